# revision 2
# baseline (speedup 1.0000x reference)
"""Single transformer block on 8 NeuronCores, v2.

Sharding: core c = (batch b=c//2, parity h=c%2). Parity-interleaved
128-token stripes; K/V computed locally for the FULL sequence (no
collectives). For h=0 cores the host swaps adjacent 128-row block pairs
of x so that own tokens sit at odd block positions on every core; the
causal structure is then parity-independent and only the diagonal-pair
mask data differs per core.

Precision: fp8e4m3 DoubleRow for QKV/AV/c_proj (weights x16), bf16 QK,
fc with hi/lo split on both operands (error-compensated), mproj with
hi/lo split weights and single-fp8 gelu activations.
"""

import math
from contextlib import ExitStack

import numpy as np
import ml_dtypes

import concourse.bacc as bacc
import concourse.bass as bass
import concourse.mybir as mybir
import concourse.tile as tile
from concourse.masks import make_identity
from concourse.tile import add_dep_helper

F32 = mybir.dt.float32
F32R = mybir.dt.float32r
BF16 = mybir.dt.bfloat16
FP8 = mybir.dt.float8e4
AF = mybir.ActivationFunctionType
ALU = mybir.AluOpType
DR = mybir.MatmulPerfMode.DoubleRow

E4 = ml_dtypes.float8_e4m3
BF = ml_dtypes.bfloat16

EPS = 1e-5
WS = 16.0


class Cfg:
    def __init__(self):
        self.B, self.S, self.D, self.H, self.F = 4, 2048, 1024, 16, 4096
        self.n_cores = 8
        self.HD = 64
        self.T = 1024
        self.DC = 8
        self.KC = 16
        self.GB = 32
        self.NSL = 8
        self.TBF = 16
        self.pqk_bufs = 2
        self.ppo_bufs = 2
        self.pmlp_bufs = 2
        self.use_deps = True
        self.defer_n = 16
        self.k_alt = True       # alternate k evictions DVE/Act
        self.ln1_act_half = True
        self.q_dve = False
        self.masks_pool = False
        self.tail_mode = 2  # 2=full overlap, 1=no fc1 defer, 0=serial


def q8(x, scale=1.0):
    x = np.asarray(x, np.float32) * scale
    return np.clip(x, -224.0, 224.0).astype(E4)


def build(cfg: Cfg):
    c = cfg
    nc = bacc.Bacc(None, target_bir_lowering=False)

    x_in = nc.dram_tensor("x", [c.S, c.D], BF16, kind="ExternalInput")
    wq_in = nc.dram_tensor("wq", [128, c.DC, c.D], FP8, kind="ExternalInput")
    wk_in = nc.dram_tensor("wk", [128, c.DC, c.D], FP8, kind="ExternalInput")
    wv_in = nc.dram_tensor("wv", [128, c.DC, c.D], FP8, kind="ExternalInput")
    wc_in = nc.dram_tensor("wc", [128, c.DC, c.D], FP8, kind="ExternalInput")
    wfc_in = nc.dram_tensor("wfc", [128, 16, 2 * c.DC, 256], FP8,
                            kind="ExternalInput")
    wmh_in = nc.dram_tensor("wmh", [128, 8, c.GB, 128], FP8, kind="ExternalInput")
    wml_in = nc.dram_tensor("wml", [128, 8, c.GB, 128], FP8, kind="ExternalInput")
    bqk_in = nc.dram_tensor("bqk", [128, 2 * c.DC], F32, kind="ExternalInput")
    bfc_in = nc.dram_tensor("bfc", [128, c.GB], F32, kind="ExternalInput")
    mk_in = nc.dram_tensor("mk", [128, 2, 128], FP8, kind="ExternalInput")
    y_out = nc.dram_tensor("y", [c.T, c.D], F32, kind="ExternalOutput")

    with tile.TileContext(nc) as tc, ExitStack() as es:
        gconst = es.enter_context(tc.tile_pool(name="gconst", bufs=1))

        identb = gconst.tile([128, 128], BF16)
        with tc.tile_pool(name="idf", bufs=1) as idf:
            identf = idf.tile([128, 128], F32)
            make_identity(nc, identf[:])
            nc.vector.tensor_copy(identb[:], identf[:])
        eps_t = gconst.tile([128, 1], F32)
        nc.vector.memset(eps_t[:], EPS)
        ones64 = gconst.tile([1, 64], F32R)
        with tc.tile_pool(name="o64", bufs=1) as o64:
            ones64f = o64.tile([1, 64], F32)
            nc.vector.memset(ones64f[:], 1.0)
            nc.vector.tensor_copy(ones64[:], ones64f[:])
        mk = gconst.tile([128, 2, 128], FP8)
        nc.sync.dma_start(out=mk[:], in_=mk_in[:, :, :])
        bqk = gconst.tile([128, 2 * c.DC], F32)
        nc.sync.dma_start(out=bqk[:], in_=bqk_in[:, :])
        bfc = gconst.tile([128, c.GB], F32)
        nc.sync.dma_start(out=bfc[:], in_=bfc_in[:, :])

        # shared PSUM pools (8 banks total across all phases):
        #   pqk: 2 bufs x 2 banks, ppo: 2 x 1 bank, pmlp: 2 x 1 bank
        pqk = es.enter_context(tc.tile_pool(name="pqk", bufs=c.pqk_bufs, space="PSUM"))
        ppo = es.enter_context(tc.tile_pool(name="ppo", bufs=c.ppo_bufs, space="PSUM"))
        pmlp = es.enter_context(tc.tile_pool(name="pmlp", bufs=c.pmlp_bufs, space="PSUM"))

        # ------------- persistent activation buffers -------------
        es_big = ExitStack()
        xpool = es_big.enter_context(tc.tile_pool(name="xo", bufs=1, side="left"))
        xt = [xpool.tile([128, c.D], BF16, tag=f"x{j}", name=f"x{j}")
              for j in range(c.NSL)]
        x2t = [xpool.tile([128, c.D], BF16, tag=f"x2_{j}", name=f"x2_{j}")
               for j in range(c.NSL)]

        es_ht = ExitStack()
        htpool = es_ht.enter_context(tc.tile_pool(name="htp", bufs=1, side="right"))
        ht = htpool.tile([128, c.DC, c.S], FP8, name="ht")

        es_kqv = ExitStack()
        kqv = es_kqv.enter_context(tc.tile_pool(name="kqv", bufs=1, side="left"))
        k_fm = [kqv.tile([128, c.S], BF16, tag=f"k{m}", name=f"k{m}")
                for m in range(c.DC)]
        q_all = kqv.tile([128, c.DC, c.T], BF16, name="q_all")
        v_all = kqv.tile([128, c.KC, c.H, 65], FP8, name="v_all")
        nc.vector.memset(v_all[:, :, :, 64:65], 1.0)

        # ============ LN1 (full seq) -> ht, QKV interleaved per th ============
        with (
            tc.tile_pool(name="xs", bufs=2) as xs,
            tc.tile_pool(name="ln1", bufs=3) as lnp,
            tc.tile_pool(name="wa", bufs=1) as wap,
        ):
            wk = wap.tile([128, c.DC, c.D], FP8, name="wk")
            wv = wap.tile([128, c.DC, c.D], FP8, name="wv")
            wq = wap.tile([128, c.DC, c.D], FP8, name="wq")
            nc.scalar.dma_start(out=wk[:], in_=wk_in[:, :, :])
            nc.scalar.dma_start(out=wv[:], in_=wv_in[:, :, :])
            nc.scalar.dma_start(out=wq[:], in_=wq_in[:, :, :])

            def ln1_tile(tb):
                if tb % 2 == 1:
                    xtile = xt[tb // 2]
                else:
                    xtile = xs.tile([128, c.D], BF16, tag="xs")
                nc.sync.dma_start(out=xtile[:],
                                  in_=x_in[tb * 128:(tb + 1) * 128, :])
                st = lnp.tile([128, 2, 6], F32, tag="st")
                for sg in range(2):
                    nc.vector.bn_stats(out=st[:, sg, :],
                                       in_=xtile[:, sg * 512:(sg + 1) * 512])
                mv = lnp.tile([128, 2], F32, tag="mv")
                nc.vector.bn_aggr(out=mv[:], in_=st[:])
                sd = lnp.tile([128, 1], F32, tag="sd")
                nc.scalar.activation(sd[:], mv[:, 1:2], AF.Sqrt,
                                     bias=eps_t[:, 0:1])
                rs = lnp.tile([128, 2], F32, tag="rs")
                nc.vector.reciprocal(rs[:, 0:1], sd[:])
                nc.vector.tensor_mul(rs[:, 1:2], mv[:, 0:1], rs[:, 0:1])
                nc.vector.tensor_scalar(out=rs[:, 1:2], in0=rs[:, 1:2],
                                        scalar1=-1.0, scalar2=None,
                                        op0=ALU.mult)
                xn = lnp.tile([128, c.D], BF16, tag="xn")
                nc.scalar.activation(xn[:], xtile[:], AF.Identity,
                                     scale=rs[:, 0:1], bias=rs[:, 1:2])
                for half in range(2):
                    ptf = pmlp.tile([128, 512], F32, tag="mlp")
                    ptr = ptf[:].bitcast(BF16).rearrange(
                        "p (a b) -> p a b", b=128)[:, 0:4, :]
                    for i in range(4):
                        dc = half * 4 + i
                        nc.tensor.transpose(
                            ptr[:, i, :], xn[:, dc * 128:(dc + 1) * 128],
                            identb[:])
                    if c.ln1_act_half and half == 1:
                        nc.scalar.activation(
                            ht[:, 4:8, tb * 128:(tb + 1) * 128], ptr[:],
                            AF.Identity)
                    else:
                        nc.vector.tensor_copy(
                            ht[:, half * 4:(half + 1) * 4,
                               tb * 128:(tb + 1) * 128], ptr[:])

            def v_pass(kc, fh):
                psf = pqk.tile([128, 2, 512], F32, tag="qk",
                               name=f"vps{kc}_{fh}")
                ps = psf[:, 0, :]
                for p in range(4):
                    nc.tensor.matmul(
                        ps,
                        ht[:, 2 * p:2 * p + 2, kc * 128:(kc + 1) * 128],
                        wv[:, 2 * p:2 * p + 2, fh * 512:(fh + 1) * 512],
                        start=(p == 0), stop=(p == 3), perf_mode=DR)
                nc.vector.tensor_scalar(
                    out=v_all[:, kc, fh * 8:(fh + 1) * 8, 0:64],
                    in0=ps.rearrange("p (h d) -> p h d", h=8),
                    scalar1=1.0 / WS, scalar2=None, op0=ALU.mult)

            def k_pass(m, th):
                ps = pqk.tile([128, 2, 512], F32, tag="qk")
                for p in range(4):
                    nc.tensor.matmul(
                        ps[:, 0, :],
                        wk[:, 2 * p:2 * p + 2, m * 128:(m + 1) * 128],
                        ht[:, 2 * p:2 * p + 2, th * 512:(th + 1) * 512],
                        start=(p == 0), stop=(p == 3), perf_mode=DR)
                if c.k_alt and m % 2 == 0:
                    nc.scalar.activation(
                        k_fm[m][:, th * 512:(th + 1) * 512], ps[:, 0, :],
                        AF.Identity, scale=1.0 / WS,
                        bias=bqk[:, c.DC + m:c.DC + m + 1])
                else:
                    nc.vector.tensor_scalar(
                        out=k_fm[m][:, th * 512:(th + 1) * 512],
                        in0=ps[:, 0, :], scalar1=1.0 / WS,
                        scalar2=bqk[:, c.DC + m:c.DC + m + 1],
                        op0=ALU.mult, op1=ALU.add)

            def q_pass(j):
                # all feature chunks for own slot j (token block 2j+1)
                psf = pqk.tile([128, 2, 512], F32, tag="qk",
                               name=f"qps{j}")
                ps = psf[:].rearrange("p two (m t) -> p (two m) t", t=128)
                ab = 2 * j + 1
                for m in range(c.DC):
                    for p in range(4):
                        nc.tensor.matmul(
                            ps[:, m, :],
                            wq[:, 2 * p:2 * p + 2, m * 128:(m + 1) * 128],
                            ht[:, 2 * p:2 * p + 2, ab * 128:(ab + 1) * 128],
                            start=(p == 0), stop=(p == 3), perf_mode=DR,
                            skip_group_check=True)
                # scale is uniform; per-m bias columns via bqk row layout
                if c.q_dve:
                    nc.vector.tensor_scalar(
                        out=q_all[:, :, j * 128:(j + 1) * 128],
                        in0=ps[:, :, :], scalar1=1.0 / (WS * 8.0),
                        scalar2=None, op0=ALU.mult)
                else:
                    nc.scalar.activation(
                        q_all[:, :, j * 128:(j + 1) * 128],
                        ps[:, :, :], AF.Identity, scale=1.0 / (WS * 8.0))

            # interleave LN1 tiles with QKV th-groups (512-token groups)
            for th in range(4):
                for tb in range(4 * th, 4 * th + 4):
                    ln1_tile(tb)
                for m in range(c.DC):
                    k_pass(m, th)
                    if m % 2 == 1:
                        kc = 4 * th + m // 2
                        v_pass(kc, 0)
                        v_pass(kc, 1)
                for j in (2 * th, 2 * th + 1):
                    q_pass(j)

        es_ht.close()

        # ================= attention + MLP, software-pipelined =================
        es_at = ExitStack()
        atp = es_at.enter_context(tc.tile_pool(name="atp", bufs=1, side="right"))
        at_all = atp.tile([128, c.DC, c.T], FP8, name="at_all")

        es_wc2 = ExitStack()
        wc2 = es_wc2.enter_context(tc.tile_pool(name="wc2", bufs=1, side="left"))
        wc = wc2.tile([128, c.DC, c.D], FP8, name="wc")
        nc.scalar.dma_start(out=wc[:], in_=wc_in[:, :, :])

        es_mlp = ExitStack()
        mtp = es_mlp.enter_context(tc.tile_pool(name="mtp", bufs=1, side="right"))
        gp = es_mlp.enter_context(tc.tile_pool(name="gp", bufs=1, side="right"))
        wmlp = es_mlp.enter_context(tc.tile_pool(name="wmlp", bufs=2, side="left"))
        ustgp = es_mlp.enter_context(tc.tile_pool(name="ustgp", bufs=1, side="left"))
        mt_h = {}
        gt_h = {}
        u_stage = {}

        def get_mt(hf):
            if hf not in mt_h:
                mt_h[hf] = mtp.tile([128, 2 * c.DC, 512], FP8, tag="mt",
                                    name=f"mt{hf}")
            return mt_h[hf]

        def get_gt(hf):
            if hf not in gt_h:
                gt_h[hf] = gp.tile([128, c.GB, 512], FP8, tag="g",
                                   name=f"g{hf}")
            return gt_h[hf]

        with (
            tc.tile_pool(name="pt", bufs=8) as ptp,
            tc.tile_pool(name="nrm", bufs=2) as nrm,
            tc.tile_pool(name="ln2", bufs=3) as ln2p,
            tc.tile_pool(name="yo", bufs=3) as yop,
        ):
            def attn_head(gi, head, act_after=None):
                exps = []

                def expop(out, in_):
                    e = nc.scalar.activation(out, in_, AF.Exp)
                    if act_after is not None:
                        add_dep_helper(e.ins, act_after.ins,
                                       reason="act table order")
                    exps.append(e)
                    return e

                jj, hp = head // 2, head % 2
                base = hp * 64
                qsl = slice(gi * 512, (gi + 1) * 512)
                n512 = 8 * gi + 2
                pts = []
                for t0 in range(0, n512, 2):
                    ps = pqk.tile([128, 2, 512], F32, tag="qk")
                    for i in range(2):
                        kc = t0 + i
                        nc.tensor.matmul(
                            ps[:, i, :],
                            k_fm[jj][base:base + 64,
                                     kc * 128:(kc + 1) * 128],
                            q_all[base:base + 64, jj, qsl],
                            start=True, stop=True, skip_group_check=True)
                    pt_ = ptp.tile([128, 2, 512], FP8, tag="pt")
                    expop(pt_[:], ps[:])
                    pts.append(pt_)
                psB = pqk.tile([128, 2, 512], F32, tag="qk")
                for i in range(2):
                    nc.tensor.matmul(
                        psB[:, i, 0:384],
                        k_fm[jj][base:base + 64,
                                 (n512 + i) * 128:(n512 + i + 1) * 128],
                        q_all[base:base + 64, jj,
                              gi * 512 + 128:(gi + 1) * 512],
                        start=True, stop=True, skip_group_check=True)
                    nc.tensor.matmul(
                        psB[:, i, 384:512],
                        k_fm[jj][base:base + 64,
                                 (n512 + 4 + i) * 128:(n512 + 5 + i) * 128],
                        q_all[base:base + 64, jj,
                              gi * 512 + 384:(gi + 1) * 512],
                        start=True, stop=True, skip_group_check=True)
                ptB = ptp.tile([128, 2, 512], FP8, tag="pt")
                expop(ptB[:], psB[:])
                psCf = pqk.tile([128, 2, 512], F32, tag="qk")
                for i in range(2):
                    nc.tensor.matmul(
                        psCf[:, i, 0:256],
                        k_fm[jj][base:base + 64,
                                 (n512 + 2 + i) * 128:(n512 + 3 + i) * 128],
                        q_all[base:base + 64, jj,
                              gi * 512 + 256:(gi + 1) * 512],
                        start=True, stop=True, skip_group_check=True)
                ptC = ptp.tile([128, 2, 256], FP8, tag="ptC")
                expop(ptC[:], psCf[:, :, 0:256])

                nc.gpsimd.tensor_tensor(
                    pts[4 * gi][:, :, 0:128], pts[4 * gi][:, :, 0:128],
                    mk[:], ALU.mult)
                mb_eng = nc.gpsimd if c.masks_pool else nc.vector
                mb_eng.tensor_tensor(
                    ptB[:, :, 0:128], ptB[:, :, 0:128], mk[:], ALU.mult)
                nc.gpsimd.tensor_tensor(
                    ptC[:, :, 0:128], ptC[:, :, 0:128], mk[:], ALU.mult)
                mb_eng.tensor_tensor(
                    ptB[:, :, 384:512], ptB[:, :, 384:512], mk[:], ALU.mult)

                po = ppo.tile([65, 4, 128], F32, tag="po")
                for jjj in range(4):
                    J = 4 * gi + jjj
                    npair = J + 1
                    csl = slice(jjj * 128, (jjj + 1) * 128)
                    for pr in range(npair):
                        kc0 = 2 * pr
                        if kc0 < n512:
                            rhs = pts[pr][:, :, csl]
                        elif kc0 == n512:
                            rhs = ptB[:, :, jjj * 128 - 128:jjj * 128]
                        elif kc0 == n512 + 2:
                            rhs = ptC[:, :, jjj * 128 - 256:jjj * 128 - 128]
                        else:
                            rhs = ptB[:, :, 384:512]
                        nc.tensor.matmul(
                            po[:, jjj, :],
                            v_all[:, kc0:kc0 + 2, head, :], rhs,
                            start=(pr == 0), stop=(pr == npair - 1),
                            perf_mode=DR, skip_group_check=True)
                rec = nrm.tile([1, 512], F32R, tag="rec")
                with nc.allow_low_precision(reason="softmax denom"):
                    nc.vector.reciprocal(
                        rec[:],
                        po[64:65, :, :].rearrange("p a b -> p (a b)"))
                bcf = pmlp.tile([128, 512], F32, tag="mlp")
                bc = bcf[0:64, :]
                nc.tensor.matmul(bc, ones64[:], rec[:], start=True,
                                 stop=True, skip_group_check=True)
                bcs = nrm.tile([64, 512], F32, tag="bcs")
                nc.vector.tensor_copy(bcs[:], bc)
                nc.vector.tensor_mul(
                    at_all[base:base + 64, jj, qsl],
                    po[0:64, :, :].rearrange("p a b -> p (a b)"),
                    bcs[:])
                return exps

            def cproj_ln2(hf, act_after=None):
                last_act = None
                for j in range(4 * hf, 4 * hf + 4):
                    x2 = x2t[j]
                    for fh in range(2):
                        fsl = slice(fh * 512, (fh + 1) * 512)
                        ps = pmlp.tile([128, 512], F32, tag="mlp")
                        for p in range(4):
                            nc.tensor.matmul(
                                ps[:],
                                at_all[:, 2 * p:2 * p + 2,
                                       j * 128:(j + 1) * 128],
                                wc[:, 2 * p:2 * p + 2, fsl],
                                start=(p == 0), stop=(p == 3), perf_mode=DR)
                        nc.vector.scalar_tensor_tensor(
                            out=x2[:, fsl], in0=ps[:], scalar=1.0 / WS,
                            in1=xt[j][:, fsl], op0=ALU.mult, op1=ALU.add)
                    st = ln2p.tile([128, 2, 6], F32, tag="st")
                    for sg in range(2):
                        nc.vector.bn_stats(
                            out=st[:, sg, :],
                            in_=x2[:, sg * 512:(sg + 1) * 512])
                    mv = ln2p.tile([128, 2], F32, tag="mv")
                    nc.vector.bn_aggr(out=mv[:], in_=st[:])
                    sd = ln2p.tile([128, 1], F32, tag="sd")
                    sq = nc.scalar.activation(sd[:], mv[:, 1:2], AF.Sqrt,
                                              bias=eps_t[:, 0:1])
                    if act_after is not None:
                        add_dep_helper(sq.ins, act_after.ins,
                                       reason="act table order")
                    rs = ln2p.tile([128, 2], F32, tag="rs")
                    nc.vector.reciprocal(rs[:, 0:1], sd[:])
                    nc.vector.tensor_mul(rs[:, 1:2], mv[:, 0:1], rs[:, 0:1])
                    nc.vector.tensor_scalar(out=rs[:, 1:2], in0=rs[:, 1:2],
                                            scalar1=-1.0, scalar2=None,
                                            op0=ALU.mult)
                    xn = ln2p.tile([128, c.D], BF16, tag="xn")
                    last_act = nc.scalar.activation(xn[:], x2[:], AF.Identity,
                                                    scale=rs[:, 0:1],
                                                    bias=rs[:, 1:2])
                    mtc = get_mt(hf)
                    mtpair = mtc[:].rearrange("p (i two) t -> p i two t",
                                              two=2)
                    tsl = slice((j - 4 * hf) * 128, (j - 4 * hf + 1) * 128)
                    for half in range(2):
                        ptf = pmlp.tile([128, 512], F32, tag="mlp")
                        ptr = ptf[:].bitcast(BF16).rearrange(
                            "p (a b) -> p a b", b=128)[:, 0:4, :]
                        for i in range(4):
                            dc = half * 4 + i
                            nc.tensor.transpose(
                                ptr[:, i, :], xn[:, dc * 128:(dc + 1) * 128],
                                identb[:])
                        nc.vector.tensor_copy(
                            mtpair[:, half * 4:(half + 1) * 4, 0, tsl],
                            ptr[:])
                        nc.vector.tensor_tensor(
                            mtpair[:, half * 4:(half + 1) * 4, 1, tsl],
                            ptr[:],
                            mtpair[:, half * 4:(half + 1) * 4, 0, tsl],
                            ALU.subtract)
                return last_act

            wfc_slab = {}

            def fc_gb(hf, gb, defer_gelu=False, act_after=None, from_qk=False):
                key = gb // 2
                if (hf, key) not in wfc_slab:
                    wt = wmlp.tile([128, 2 * c.DC, 256], FP8, tag="wfc",
                                   name=f"wfc{hf}_{key}")
                    nc.sync.dma_start(
                        out=wt[:],
                        in_=wfc_in[:, key, :, :])
                    wfc_slab[(hf, key)] = wt
                wfc = wfc_slab[(hf, key)]
                wfc_p = wfc[:].rearrange("p (i two) f -> p i two f", two=2)
                mtc = get_mt(hf)
                mtpair = mtc[:].rearrange("p (i two) t -> p i two t", two=2)
                gcol = (gb % 2) * 128
                gsl = slice(gcol, gcol + 128)
                if from_qk:
                    psf = pqk.tile([128, 2, 512], F32, tag="qk",
                                   name=f"fcq{hf}_{gb}")
                    ps = psf[:, 0, :]
                else:
                    psf = pmlp.tile([128, 512], F32, tag="mlp",
                                    name=f"fcm{hf}_{gb}")
                    ps = psf[:]
                for pp_ in range(4):
                    nc.tensor.matmul(
                        ps, wfc_p[:, 2 * pp_:2 * pp_ + 2, 1, gsl],
                        mtpair[:, 2 * pp_:2 * pp_ + 2, 0, :],
                        start=(pp_ == 0), stop=False, perf_mode=DR)
                for ci in range(c.DC):
                    nc.tensor.matmul(
                        ps, wfc[:, 2 * ci:2 * ci + 2, gsl],
                        mtc[:, 2 * ci:2 * ci + 2, :],
                        start=False, stop=(ci == c.DC - 1), perf_mode=DR)
                if defer_gelu:
                    # DVE eviction keeps the Act engine free (pure exp) in
                    # the attention overlap window; gelu batched later
                    if hf not in u_stage:
                        u_stage[hf] = ustgp.tile([128, 16, 512], BF16,
                                                 tag="ustg", name=f"u{hf}")
                    nc.vector.tensor_scalar(
                        out=u_stage[hf][:, gb, :], in0=ps,
                        scalar1=1.0 / WS, scalar2=bfc[:, gb:gb + 1],
                        op0=ALU.mult, op1=ALU.add)
                else:
                    ge = nc.scalar.activation(
                        get_gt(hf)[:, gb, :], ps, AF.Gelu_apprx_tanh,
                        scale=1.0 / WS, bias=bfc[:, gb:gb + 1])
                    if act_after is not None:
                        add_dep_helper(ge.ins, act_after.ins,
                                       reason="act table order")
                    return ge

            wm_slab = {}

            def wm_prefetch(hf, fq):
                if (hf, fq) not in wm_slab:
                    th_ = wmlp.tile([128, c.GB, 128], FP8, tag="wmh",
                                    name=f"wmh{hf}_{fq}")
                    tl_ = wmlp.tile([128, c.GB, 128], FP8, tag="wml",
                                    name=f"wml{hf}_{fq}")
                    nc.scalar.dma_start(out=th_[:], in_=wmh_in[:, fq, :, :])
                    nc.scalar.dma_start(out=tl_[:], in_=wml_in[:, fq, :, :])
                    wm_slab[(hf, fq)] = (th_, tl_)

            def mproj_tile(hf, tb, fq):
                # out tile: tokens tb*128, D-eighth fq (128 wide)
                wm_prefetch(hf, fq)
                wmh, wml = wm_slab[(hf, fq)]
                g = get_gt(hf)
                tloc = slice((tb - 4 * hf) * 128, (tb - 4 * hf + 1) * 128)
                fsl = slice(fq * 128, (fq + 1) * 128)
                psf = pmlp.tile([128, 512], F32, tag="mlp")
                ps = psf[:, 0:128]
                for pr in range(c.GB // 2):
                    nc.tensor.matmul(
                        ps, g[:, 2 * pr:2 * pr + 2, tloc],
                        wmh[:, 2 * pr:2 * pr + 2, :],
                        start=(pr == 0), stop=False, perf_mode=DR)
                for pr in range(c.GB // 2):
                    nc.tensor.matmul(
                        ps, g[:, 2 * pr:2 * pr + 2, tloc],
                        wml[:, 2 * pr:2 * pr + 2, :],
                        start=False, stop=(pr == c.GB // 2 - 1), perf_mode=DR)
                yo = yop.tile([128, 128], F32, tag="yo")
                nc.vector.scalar_tensor_tensor(
                    out=yo[:], in0=ps, scalar=1.0 / WS,
                    in1=x2t[tb][:, fsl], op0=ALU.mult, op1=ALU.add)
                nc.scalar.dma_start(out=y_out[tb * 128:(tb + 1) * 128, fsl],
                                   in_=yo[:])

            # ---------------- schedule ----------------
            for head in range(c.H):
                last_exps = attn_head(0, head)
            ln2_0_last = cproj_ln2(0, act_after=last_exps[-1] if c.use_deps else None)
            wm_prefetch(0, 0)
            wm_prefetch(0, 1)
            # attn(gi=1) interleaved with fc(0) gb0-15; Act stays on the exp
            # table through the window (fc evictions ride on DVE)
            fc_done = 0
            for head in range(c.H):
                last_exps = attn_head(1, head, act_after=ln2_0_last if c.use_deps else None)
                while fc_done < min(c.defer_n, (head + 1) * 2):
                    fc_gb(0, fc_done, defer_gelu=True)
                    fc_done += 1
            last_exp1 = last_exps[-1]

            def gelu_batch(hf, n0, n1, dep):
                last = None
                for gb in range(n0, n1):
                    ge = nc.scalar.activation(get_gt(hf)[:, gb, :],
                                              u_stage[hf][:, gb, :],
                                              AF.Gelu_apprx_tanh)
                    if c.use_deps and dep is not None:
                        add_dep_helper(ge.ins, dep.ins,
                                       reason="act table order")
                    last = ge
                return last

            last_gelu = gelu_batch(0, 0, c.defer_n, last_exp1)
            for gb in range(c.defer_n, c.GB):
                last_gelu = fc_gb(0, gb, act_after=last_exp1 if c.use_deps else None)
            if c.tail_mode == 2:
                ln2_1_last = cproj_ln2(1, act_after=last_gelu if c.use_deps else None)
                # overlap mproj(0) with deferred-evict fc(1) gb0-15
                fc1 = 0
                for mp in range(32):
                    mproj_tile(0, mp % 4, mp // 4)
                    if mp % 2 == 1 and fc1 < 16:
                        fc_gb(1, fc1, defer_gelu=True, from_qk=True)
                        fc1 += 1
                last_gelu1 = gelu_batch(1, 0, 16, ln2_1_last)
                for gb in range(16, c.GB):
                    last_gelu1 = fc_gb(1, gb,
                                       act_after=ln2_1_last if c.use_deps else None,
                                       from_qk=True)
            elif c.tail_mode == 1:
                ln2_1_last = cproj_ln2(1, act_after=last_gelu if c.use_deps else None)
                for mp in range(32):
                    mproj_tile(0, mp % 4, mp // 4)
                for gb in range(c.GB):
                    fc_gb(1, gb, act_after=None, from_qk=True)
            else:
                for mp in range(32):
                    mproj_tile(0, mp % 4, mp // 4)
                ln2_1_last = cproj_ln2(1, act_after=None)
                for gb in range(c.GB):
                    fc_gb(1, gb, act_after=None)
            for fq in range(8):
                for tb in range(4, 8):
                    mproj_tile(1, tb, fq)

        es_mlp.close()
        es_wc2.close()
        es_at.close()
        es_kqv.close()
        es_big.close()

    nc.compile()
    return nc


_NC_CACHE = {}


def get_nc(cfg=None):
    if "nc" not in _NC_CACHE:
        _NC_CACHE["nc"] = build(cfg or Cfg())
    return _NC_CACHE["nc"]


def make_core_inputs(cfg, x, ln1_w, ln1_b, W_attn, b_attn, W_cproj, b_cproj,
                     ln2_w, ln2_b, W_fc, b_fc, W_mproj, b_mproj):
    c = cfg
    f32 = np.float32
    x = np.asarray(x, f32)
    ln1_w = np.asarray(ln1_w, f32)
    ln1_b = np.asarray(ln1_b, f32)
    ln2_w = np.asarray(ln2_w, f32)
    ln2_b = np.asarray(ln2_b, f32)
    W_attn = np.asarray(W_attn, f32)
    W_cproj = np.asarray(W_cproj, f32)
    W_fc = np.asarray(W_fc, f32)
    W_mproj = np.asarray(W_mproj, f32)
    b_attn = np.asarray(b_attn, f32)

    Wa = ln1_w[:, None] * W_attn
    ba = b_attn + ln1_b @ W_attn
    Wf = ln2_w[:, None] * W_fc
    bf2 = np.asarray(b_fc, f32) + ln2_b @ W_fc
    assert not np.any(np.asarray(b_cproj, f32)), "nonzero b_cproj unsupported"
    assert not np.any(np.asarray(b_mproj, f32)), "nonzero b_mproj unsupported"
    assert not np.any(ba[2 * c.D:]), "nonzero v bias unsupported"
    assert not np.any(ba[0:c.D]), "nonzero q bias unsupported"

    def wlay(w, scale=WS):
        dd, ff = w.shape
        return np.ascontiguousarray(
            q8(w, scale).reshape(dd // 128, 128, ff).transpose(1, 0, 2))

    wq_ = wlay(Wa[:, 0:c.D])
    wk_ = wlay(Wa[:, c.D:2 * c.D])
    wv_ = wlay(Wa[:, 2 * c.D:3 * c.D])
    wc_ = wlay(W_cproj)

    Wf16 = Wf * WS
    wf_hi8 = q8(Wf16, 1.0)
    wf_lo8 = q8(Wf16 - wf_hi8.astype(f32), 1.0)
    wfc = np.empty((128, 2 * c.DC, c.F), E4)
    wfc[:, 1::2, :] = wf_hi8.reshape(c.DC, 128, c.F).transpose(1, 0, 2)
    wfc[:, 0::2, :] = wf_lo8.reshape(c.DC, 128, c.F).transpose(1, 0, 2)
    # slab-contiguous: [128, key, 2*DC, 256]
    wfc = np.ascontiguousarray(
        wfc.reshape(128, 2 * c.DC, 16, 256).transpose(0, 2, 1, 3))

    Wm16 = W_mproj * WS
    wm_hi8 = q8(Wm16, 1.0)
    wm_lo8 = q8(Wm16 - wm_hi8.astype(f32), 1.0)
    # slab-contiguous: [128, fq, GB, 128]
    wmh = np.ascontiguousarray(
        wm_hi8.reshape(c.GB, 128, 8, 128).transpose(1, 2, 0, 3))
    wml = np.ascontiguousarray(
        wm_lo8.reshape(c.GB, 128, 8, 128).transpose(1, 2, 0, 3))

    bqk = np.zeros((128, 2 * c.DC), f32)
    bqk[:, 0:c.DC] = (ba[0:c.D] / 8.0).reshape(c.DC, 128).T
    bqk[:, c.DC:] = ba[c.D:2 * c.D].reshape(c.DC, 128).T
    bfc_l = np.ascontiguousarray(bf2.reshape(c.GB, 128).T)

    shared = {
        "wq": wq_, "wk": wk_, "wv": wv_, "wc": wc_, "wfc": wfc,
        "wmh": wmh, "wml": wml, "bqk": bqk, "bfc": bfc_l,
    }

    r = np.arange(128)
    tri = (r[None, :] >= r[:, None]).astype(f32)  # [k, q]: pass iff q >= k
    mk0 = np.stack([np.zeros((128, 128), f32), tri], axis=1)
    mk1 = np.stack([np.ones((128, 128), f32), tri], axis=1)
    swap = np.arange(16).reshape(8, 2)[:, ::-1].reshape(16)
    in_maps = []
    for core in range(c.n_cores):
        b, h = core // 2, core % 2
        m = dict(shared)
        xb = x[b].reshape(16, 128, c.D)
        if h == 0:
            xb = xb[swap]
        m["x"] = np.ascontiguousarray(xb.reshape(c.S, c.D).astype(BF))
        m["mk"] = (mk0 if h == 0 else mk1).astype(E4)
        in_maps.append(m)
    return in_maps


def core_rows(cfg, h):
    j = np.arange(cfg.T)
    return (2 * (j // 128) + h) * 128 + j % 128


def kernel(**inputs) -> np.ndarray:
    from concourse.bass_utils import run_bass_kernel_spmd

    cfg = Cfg()
    nc = get_nc(cfg)
    in_maps = make_core_inputs(cfg, **inputs)
    res = run_bass_kernel_spmd(nc, in_maps, core_ids=list(range(cfg.n_cores)))
    B = cfg.B
    out = np.empty((B, cfg.S, cfg.D), np.float32)
    for core in range(cfg.n_cores):
        b, h = core // 2, core % 2
        out[b, core_rows(cfg, h), :] = res.results[core]["y"]
    return out
